# revision 43
# baseline (speedup 1.0000x reference)
"""Trainium2 Bass kernel for nn_CombinedTargetIOULoss (B=64, K=17, H=W=64).

Data-parallel over batch: 8 cores x 8 batches each. Per core the kernel
computes per-(b,k) partial sums [sum(q1+q2), sum((hp-hg)^2)] over the 4096
pixels; the host combines them into the scalar loss (incl. target-weight
scaling and the tw==0 mask case).

Key algebra (pixel anchors cancel out of the reference box math):
  dx = |p-g|, sx = |p|+|g|; u2 = sx-dx = 2*iw, v2 = sx+dx = 2*cw (same y)
  inter = it4/4 (it4 = u2x*u2y), area_c = ac4/4, t1 = |p||q|, t2 = |g||h|
  union = t1+t2+EPS-it4/4;  q1 = inter/union, q2 = union/area_c
  Scaled to dodge divisions by 4:
    ue4 = 4(t1+t2+EPS) - it4        -> q1 = it4 * (1/ue4)
    ac4e = ac4 + 4*EPS              -> q2 = ue4 * (1/ac4e)
  (ue4 >= 2(t1+t2) since inter <= (area_p+area_g)/2 -> no cancellation.)

Layout: partition = (b, pc) with pc = p//256; the host pre-transposes the
inputs to [b, pc, kc, s] so every per-chunk DMA is a fully contiguous
[128, kg*3*256] block (multi-KB descriptors; the old 256B descriptors paid
the <512B 2x DMA latency penalty). Components (hm/ox/oy) sit at different
free offsets of the same partition, so cross-component elementwise ops
stay partition-aligned.

Engine split (ASSIGN below; DMA ~37-41us/core is the roofline):
  Pool: raw f32 diff + the sxy/u2/v2 box sums (tensor_tensor)
  ACT : |ox|,|oy| of both tensors, |diff_xy|, diff_hm^2 (unary, any dtype)
  DVE : bf16 products (tensor_tensor = 2x perf mode), the two
        RECIPROCAL_APPROX_FAST, and the tensor_scalar affines (4x mode)
  PE  : pixel reduction: 16 psum-accumulating one-hot matmuls per
        (tensor, chunk) fold s and (b,pc)->b at once; PE is otherwise idle
All cross-engine sync is derived automatically by a writer/reader tracker
(standalone wait_ge on monotone per-engine counters; per-DMA semaphores
because DMA completion order across transfers is not guaranteed).
"""

import sys

sys.path.insert(0, "/opt/trn_rl_repo")

import numpy as np

import concourse.bass as bass
from concourse import mybir
from concourse.alu_op_type import AluOpType as Alu

F32 = mybir.dt.float32
BF16 = mybir.dt.bfloat16
AF = mybir.ActivationFunctionType

EPS = 1e-7
B, K, H, W = 64, 17, 64, 64
C = 3 * K
P = H * W
N_CORES = 8
B_LOC = B // N_CORES          # 8 batches per core
PC = 16                       # pixel chunks per batch -> partition = (b, pc)
S = P // PC                   # 256 pixels per partition line per (k, c)
NSI = 16                      # matmul si-slices folding s

CH = [2, 4, 4, 4, 2, 1]       # k-group chunk sizes (sum = 17)
KMAX = max(CH)
NCH = len(CH)
# SWDGE (gpsimd-dispatched) DMA casts f32->bf16 in flight: halves SBUF
# write traffic and makes every eng op see 2-byte operands (DVE 2x/4x).
CAST_DMA = True

# op -> engine stream ('gp' = Pool, 'act' = Scalar, 'dve' = Vector)
ASSIGN = {
    "diff": "dve", "sxy": "gp", "u2": "dve", "v2": "dve",
    "oabs": "act", "tabs": "act", "dxy": "act", "dsq": "act",
    "x1m": "dve", "ac4e": "dve",
    "t1": "dve", "t2": "dve", "it4": "dve", "ac4": "dve", "x1": "dve",
    "ue4": "dve", "m": "dve", "n1": "dve", "n2": "dve", "n": "dve",
    "qsP": "dve", "tr1": "gp", "tr2": "gp", "tr3": "dve", "tr4": "dve",
}


class _Waiter:
    """Dedupe monotone standalone waits per (engine, sem)."""

    def __init__(self):
        self.seen = {}

    def wait(self, eng, sem, val):
        if val <= 0:
            return
        key = (id(eng), sem.name if hasattr(sem, "name") else id(sem))
        if self.seen.get(key, -1) >= val:
            return
        self.seen[key] = val
        eng.wait_ge(sem, val)


class _Tracker:
    """Auto-emit cross-engine waits from tile writer/reader records.

    Tiles are slot-granular keys. Same-stream dependencies rely on the
    engine's in-order execution and emit nothing.
    """

    def __init__(self, wt):
        self.wt = wt
        self.tiles = {}   # key -> {"w": (stream, sem, cnt) | None,
                          #         "r": {stream: (sem, cnt)}}

    def _rec(self, key):
        return self.tiles.setdefault(key, {"w": None, "r": {}})

    def pre(self, stream, eng, ins, outs):
        for key in ins:
            w = self._rec(key)["w"]
            if w and w[0] != stream:
                self.wt.wait(eng, w[1], w[2])
        for key in outs:
            rec = self._rec(key)
            w = rec["w"]
            if w and w[0] != stream:
                self.wt.wait(eng, w[1], w[2])
            for rs, (rsem, rcnt) in rec["r"].items():
                if rs != stream:
                    self.wt.wait(eng, rsem, rcnt)

    def post(self, stream, sem, cnt, ins, outs):
        for key in ins:
            self._rec(key)["r"][stream] = (sem, cnt)
        for key in outs:
            rec = self._rec(key)
            rec["w"] = (stream, sem, cnt)
            rec["r"] = {}


def _build_body(nc, o_ext, t_ext, w_ext, p_ext):
    from concourse.dve_ops import (
        RECIP_APPROX_FAST_CONSTS as RAF_C,
        RECIPROCAL_APPROX_FAST as RAF_OP,
    )

    sb = lambda name, shape, dt: nc.alloc_sbuf_tensor(name, shape, dt).ap()

    ov = o_ext.rearrange("b pc kc s -> (b pc) (kc s)")
    tv = t_ext.rearrange("b pc kc s -> (b pc) (kc s)")

    # --- SBUF tiles, all double-buffered by chunk parity ---
    def pair(name, ncomp, dt=BF16, s=S):
        ts = [sb(f"{name}{i}", [128, KMAX * ncomp * s], dt) for i in range(2)]
        return [t.rearrange("p (k c s) -> p k c s", k=KMAX, c=ncomp, s=s)
                for t in ts]

    IND = BF16 if CAST_DMA else F32
    in2 = [sb(f"in2{i}", [128, 2 * KMAX * 3 * S], IND) for i in range(2)]
    in2v = [t.rearrange("p (t k c s) -> p t k c s", t=2, k=KMAX, c=3, s=S)
            for t in in2]
    in2f = [t.rearrange("p (t x) -> p t x", t=2, x=KMAX * 3 * S) for t in in2]
    diffv = pair("df", 3)
    abov = pair("abo", 2)
    abtv = pair("abt", 2)
    dxyv = pair("dxy", 2)
    sxyv = pair("sxy", 2)
    u2v = pair("u2", 2)
    v2v = pair("v2", 2)
    one = lambda name: [t[:, :, 0] for t in pair(name, 1)]
    t1v, t2v, it4v, ac4v, x1v, x1mv, ue4v, ac4ev, mv, rv, n1v, n2v, nv = (
        one(n) for n in ("t1", "t2", "it4", "ac4", "x1", "x1m", "ue4",
                         "ac4e", "mT", "rT", "n1", "n2", "nT"))
    # Q stacks {qsP, dsq} as (k, t, s) so the PE stage sees (k, t) slices
    # that flatten contiguously; tree scratch folds s 256 -> 32
    Qts = [sb(f"Q{i}", [128, KMAX * 2 * S], BF16) for i in range(2)]
    Qv = [t.rearrange("p (k t s) -> p k t s", k=KMAX, t=2, s=S) for t in Qts]
    tAs = [sb(f"tA{i}", [128, KMAX * 2 * 128], BF16) for i in range(2)]
    tAv = [t.rearrange("p (k t s) -> p k t s", k=KMAX, t=2, s=128) for t in tAs]
    tBs = [sb(f"tB{i}", [128, KMAX * 2 * 64], BF16) for i in range(2)]
    tBv = [t.rearrange("p (k t s) -> p k t s", k=KMAX, t=2, s=64) for t in tBs]
    Q32s = [sb(f"Q32{i}", [128, KMAX * 2 * 32], BF16) for i in range(2)]
    Q32v = [t.rearrange("p (k t s) -> p k t s", k=KMAX, t=2, s=32)
            for t in Q32s]
    Q16s = [sb(f"Q16{i}", [128, KMAX * 2 * 16], BF16) for i in range(2)]
    Q16v = [t.rearrange("p (k t s) -> p k t s", k=KMAX, t=2, s=16)
            for t in Q16s]

    wts = sb("wts", [128, B_LOC], BF16)
    osb = sb("osb", [B_LOC, 2 * K], F32)
    psum = nc.alloc_psum_tensor("ps", [B_LOC, 2 * K], F32).ap()
    psumv = psum.rearrange("p (k t) -> p k t", k=K, t=2)

    # --- semaphores ---
    dma_o = [nc.alloc_semaphore(f"dma_o{j}") for j in range(NCH)]
    dma_t = [nc.alloc_semaphore(f"dma_t{j}") for j in range(NCH)]
    wdma = nc.alloc_semaphore("wdma")
    dma_out = nc.alloc_semaphore("dma_out")
    act_c = nc.alloc_semaphore("act_c")
    dve_c = nc.alloc_semaphore("dve_c")
    gp_c = nc.alloc_semaphore("gp_c")
    pe_c = nc.alloc_semaphore("pe_c")
    wt = _Waiter()
    tk = _Tracker(wt)

    ENG = {"gp": (nc.gpsimd, gp_c), "act": (nc.scalar, act_c),
           "dve": (nc.vector, dve_c), "pe": (nc.tensor, pe_c)}
    cnt = {"gp": 0, "act": 0, "dve": 0, "pe": 0}

    def run(stream, ins, outs, emit):
        eng, sem = ENG[stream]
        tk.pre(stream, eng, ins, outs)
        emit(eng).then_inc(sem, 1)
        cnt[stream] += 1
        tk.post(stream, sem, cnt[stream], ins, outs)

    # binary op via the op->engine table
    def tt(name, out_ap, out_key, a_ap, a_key, b_ap, b_key, op):
        st = ASSIGN[name]
        run(st, [a_key, b_key], [out_key],
            lambda eng: eng.tensor_tensor(out_ap, a_ap, b_ap, op))

    # unary: ACT activation or DVE tensor_scalar (4x perf mode)
    def un(name, out_ap, out_key, in_ap, in_key, func, ts_args, **kw):
        st = ASSIGN[name]
        if st == "act":
            run(st, [in_key], [out_key],
                lambda eng: nc.scalar.activation(out_ap, in_ap, func, **kw))
        else:
            s1, s2, op0, op1 = ts_args
            if op1 is Ellipsis:
                run("dve", [in_key], [out_key],
                    lambda eng: eng.tensor_scalar(out_ap, in_ap, s1, s2, op0))
            else:
                run("dve", [in_key], [out_key],
                    lambda eng: eng.tensor_scalar(out_ap, in_ap, s1, s2,
                                                  op0, op1))

    # --- one-hot PE weights: SWDGE (gpsimd) DMA casts f32 -> bf16 ---
    nc.gpsimd.dma_start(out=wts[:], in_=w_ext[:]).then_inc(wdma, 16)
    tk.post("dma", wdma, 16, [], [("wts",)])

    K0 = [sum(CH[:i]) for i in range(NCH)]

    dma_eng, dma_stream = ((nc.gpsimd, "gp") if CAST_DMA
                           else (nc.sync, "sync"))

    def emit_dma(j):
        kg, k0 = CH[j], K0[j]
        sl = j % 2
        cs0, csn = 3 * k0 * S, 3 * kg * S
        tk.pre(dma_stream, dma_eng, [], [("ino", sl)])
        dma_eng.dma_start(
            out=in2f[sl][:, 0, 0:csn], in_=ov[:, cs0:cs0 + csn]
        ).then_inc(dma_o[j], 16)
        tk.post("dma", dma_o[j], 16, [], [("ino", sl)])
        tk.pre(dma_stream, dma_eng, [], [("int", sl)])
        dma_eng.dma_start(
            out=in2f[sl][:, 1, 0:csn], in_=tv[:, cs0:cs0 + csn]
        ).then_inc(dma_t[j], 16)
        tk.post("dma", dma_t[j], 16, [], [("int", sl)])

    def emit_diff(j):
        kg, sl = CH[j], j % 2
        tt("diff", diffv[sl][:, 0:kg], ("df", sl),
           in2v[sl][:, 0, 0:kg], ("ino", sl),
           in2v[sl][:, 1, 0:kg], ("int", sl), Alu.subtract)

    def emit_abs(j):
        kg, sl = CH[j], j % 2
        un("oabs", abov[sl][:, 0:kg], ("abo", sl),
           in2v[sl][:, 0, 0:kg, 1:3], ("ino", sl), AF.Abs,
           (0.0, None, Alu.abs_max, Ellipsis))
        un("tabs", abtv[sl][:, 0:kg], ("abt", sl),
           in2v[sl][:, 1, 0:kg, 1:3], ("int", sl), AF.Abs,
           (0.0, None, Alu.abs_max, Ellipsis))
        un("dxy", dxyv[sl][:, 0:kg], ("dxy", sl),
           diffv[sl][:, 0:kg, 1:3], ("df", sl), AF.Abs,
           (0.0, None, Alu.abs_max, Ellipsis))
        if ASSIGN["dsq"] == "act":
            un("dsq", Qv[sl][:, 0:kg, 1], ("Q", sl),
               diffv[sl][:, 0:kg, 0], ("df", sl), AF.Square,
               (None, None, None, None))
        else:
            tt("dsq", Qv[sl][:, 0:kg, 1], ("Q", sl),
               diffv[sl][:, 0:kg, 0], ("df", sl),
               diffv[sl][:, 0:kg, 0], ("df", sl), Alu.mult)

    def emit_box(j):
        kg, sl = CH[j], j % 2
        tt("sxy", sxyv[sl][:, 0:kg], ("sxy", sl),
           abov[sl][:, 0:kg], ("abo", sl),
           abtv[sl][:, 0:kg], ("abt", sl), Alu.add)
        tt("u2", u2v[sl][:, 0:kg], ("u2", sl),
           sxyv[sl][:, 0:kg], ("sxy", sl),
           dxyv[sl][:, 0:kg], ("dxy", sl), Alu.subtract)
        tt("v2", v2v[sl][:, 0:kg], ("v2", sl),
           sxyv[sl][:, 0:kg], ("sxy", sl),
           dxyv[sl][:, 0:kg], ("dxy", sl), Alu.add)

    def emit_dve_a(j):
        kg, sl = CH[j], j % 2
        tt("t1", t1v[sl][:, 0:kg], ("t1", sl),
           abov[sl][:, 0:kg, 0], ("abo", sl),
           abov[sl][:, 0:kg, 1], ("abo", sl), Alu.mult)
        tt("t2", t2v[sl][:, 0:kg], ("t2", sl),
           abtv[sl][:, 0:kg, 0], ("abt", sl),
           abtv[sl][:, 0:kg, 1], ("abt", sl), Alu.mult)
        tt("it4", it4v[sl][:, 0:kg], ("it4", sl),
           u2v[sl][:, 0:kg, 0], ("u2", sl),
           u2v[sl][:, 0:kg, 1], ("u2", sl), Alu.mult)
        tt("ac4", ac4v[sl][:, 0:kg], ("ac4t", sl),
           v2v[sl][:, 0:kg, 0], ("v2", sl),
           v2v[sl][:, 0:kg, 1], ("v2", sl), Alu.mult)
        tt("x1", x1v[sl][:, 0:kg], ("x1", sl),
           t1v[sl][:, 0:kg], ("t1", sl),
           t2v[sl][:, 0:kg], ("t2", sl), Alu.add)

    def emit_act_affine(j):
        kg, sl = CH[j], j % 2
        # ac4e = ac4 + 4eps;  x1m = 4*x1 + 4eps (= 4(t1+t2+eps))
        un("ac4e", ac4ev[sl][:, 0:kg], ("ac4e", sl),
           ac4v[sl][:, 0:kg], ("ac4t", sl), AF.Copy,
           (1.0, 4.0 * EPS, Alu.mult, Alu.add), scale=1.0, bias=4.0 * EPS)
        un("x1m", x1mv[sl][:, 0:kg], ("x1m", sl),
           x1v[sl][:, 0:kg], ("x1", sl), AF.Copy,
           (4.0, 4.0 * EPS, Alu.mult, Alu.add), scale=4.0, bias=4.0 * EPS)

    def emit_dve_b(j):
        kg, sl = CH[j], j % 2
        # qs + 1 = (x1m*ac4e + ue4^2) / (ue4*ac4e); host subtracts the +1
        tt("ue4", ue4v[sl][:, 0:kg], ("ue4", sl),
           x1mv[sl][:, 0:kg], ("x1m", sl),
           it4v[sl][:, 0:kg], ("it4", sl), Alu.subtract)
        tt("m", mv[sl][:, 0:kg], ("m", sl),
           ue4v[sl][:, 0:kg], ("ue4", sl),
           ac4ev[sl][:, 0:kg], ("ac4e", sl), Alu.mult)
        run("dve", [("m", sl)], [("r", sl)],
            lambda eng: eng._custom_dve(
                RAF_OP, out=rv[sl][:, 0:kg], in0=mv[sl][:, 0:kg],
                s0=RAF_C["s0"], s1=RAF_C["s1"], imm2=RAF_C["imm2"]))
        tt("n1", n1v[sl][:, 0:kg], ("n1", sl),
           x1mv[sl][:, 0:kg], ("x1m", sl),
           ac4ev[sl][:, 0:kg], ("ac4e", sl), Alu.mult)
        tt("n2", n2v[sl][:, 0:kg], ("n2", sl),
           ue4v[sl][:, 0:kg], ("ue4", sl),
           ue4v[sl][:, 0:kg], ("ue4", sl), Alu.mult)
        tt("n", nv[sl][:, 0:kg], ("n", sl),
           n1v[sl][:, 0:kg], ("n1", sl),
           n2v[sl][:, 0:kg], ("n2", sl), Alu.add)
        tt("qsP", Qv[sl][:, 0:kg, 0], ("Q", sl),
           nv[sl][:, 0:kg], ("n", sl),
           rv[sl][:, 0:kg], ("r", sl), Alu.mult)
        # pairwise tree folds s 256 -> 32 for both Q halves at once
        tt("tr1", tAv[sl][:, 0:kg], ("tA", sl),
           Qv[sl][:, 0:kg, :, 0:128], ("Q", sl),
           Qv[sl][:, 0:kg, :, 128:256], ("Q", sl), Alu.add)
        tt("tr2", tBv[sl][:, 0:kg], ("tB", sl),
           tAv[sl][:, 0:kg, :, 0:64], ("tA", sl),
           tAv[sl][:, 0:kg, :, 64:128], ("tA", sl), Alu.add)
        tt("tr3", Q32v[sl][:, 0:kg], ("Q32", sl),
           tBv[sl][:, 0:kg, :, 0:32], ("tB", sl),
           tBv[sl][:, 0:kg, :, 32:64], ("tB", sl), Alu.add)
        tt("tr4", Q16v[sl][:, 0:kg], ("Q16", sl),
           Q32v[sl][:, 0:kg, :, 0:16], ("Q32", sl),
           Q32v[sl][:, 0:kg, :, 16:32], ("Q32", sl), Alu.add)

    def emit_pe(j):
        kg, k0, sl = CH[j], K0[j], j % 2
        # fold the remaining s=32 and (b,pc)->b at once: 32 matmuls
        # accumulate psum[:, :, k0:k1] += wts^T @ Q32[:, :, :, si]
        eng, sem = ENG["pe"]
        tk.pre("pe", eng, [("Q16", sl), ("wts",)], [])
        for si in range(16):
            nc.tensor.matmul(
                psumv[:, k0:k0 + kg], wts[:],
                Q16v[sl][:, 0:kg, :, si],
                start=(si == 0), stop=(si == 15),
            ).then_inc(pe_c, 1)
            cnt["pe"] += 1
        tk.post("pe", sem, cnt["pe"], [("Q16", sl), ("wts",)], [])

    # --- software-pipelined emission (1-chunk skew; diff_j sits mid-way
    # through the j-1 DVE group so DVE never head-blocks on fresh DMA) ---
    for j in range(NCH):
        emit_dma(j)
        if j >= 1:
            emit_box(j - 1)
            emit_dve_a(j - 1)
        emit_diff(j)
        if j >= 1:
            emit_act_affine(j - 1)
            emit_dve_b(j - 1)
            emit_pe(j - 1)
        emit_abs(j)
    emit_box(NCH - 1)
    emit_dve_a(NCH - 1)
    emit_act_affine(NCH - 1)
    emit_dve_b(NCH - 1)
    emit_pe(NCH - 1)

    # --- tail: psum -> sbuf -> dram ---
    wt.wait(nc.vector, pe_c, cnt["pe"])
    run("dve", [], [("osb",)],
        lambda eng: eng.tensor_copy(osb[:], psum[:]))
    tk.pre("sync", nc.sync, [("osb",)], [])
    nc.sync.dma_start(out=p_ext[:], in_=osb[:]).then_inc(dma_out, 16)
    nc.sync.wait_ge(dma_out, 16)


def build_nc():
    nc = bass.Bass()
    o_ext = nc.declare_dram_parameter("output", [B_LOC, PC, C, S], F32,
                                      isOutput=False)
    t_ext = nc.declare_dram_parameter("target", [B_LOC, PC, C, S], F32,
                                      isOutput=False)
    w_ext = nc.declare_dram_parameter("wconst", [128, B_LOC], F32,
                                      isOutput=False)
    p_ext = nc.declare_dram_parameter("partials", [B_LOC, 2 * K], F32,
                                      isOutput=True)
    _build_body(nc, o_ext, t_ext, w_ext, p_ext)
    # fill the 64-byte ISA encodings of custom DVE ops (reciprocal_approx):
    # Bacc.compile() does this; the raw-Bass + PJRT path does not.
    mybir.codegen_inst_isa_subclasses(nc)
    return nc


_NC = None


def _get_nc():
    global _NC
    if _NC is None:
        _NC = build_nc()
    return _NC


def _combine(parts, target_weights):
    """parts: [8 cores, 8, 34] f32 -> scalar loss (host-side finish)."""
    arr = np.asarray(parts, np.float64).reshape(B, K, 2)
    sqs = arr[:, :, 0]      # sum over pixels of (q1 + q2 + 1), per (b, k)
    ssd = arr[:, :, 1]      # sum over pixels of (hp - hg)^2, per (b, k)

    tw = np.asarray(target_weights, np.float64)
    twnz = (tw != 0).astype(np.float64)
    # kernel stores sum(q1+q2+1) so the giou term is 2P - (sqs - P)
    num = ((3.0 * P - sqs) * twnz).sum(axis=0)
    den = np.maximum((P * twnz).sum(axis=0), 1.0)
    giou_joint = num / den
    mse = 0.5 * (tw**2 * ssd).sum(axis=0) / (B * P)
    return np.float32(np.sum(mse + giou_joint) / K)


def make_in_maps(output, target):
    """Host-side relayout [B, C, P] -> [B, pc, C, s] (contiguous DMA blocks)
    plus the one-hot fold matrix, split per core."""
    output = np.ascontiguousarray(
        np.asarray(output, dtype=np.float32).reshape(B, C, PC, S)
        .transpose(0, 2, 1, 3))
    target = np.ascontiguousarray(
        np.asarray(target, dtype=np.float32).reshape(B, C, PC, S)
        .transpose(0, 2, 1, 3))
    wconst = np.repeat(np.eye(B_LOC, dtype=np.float32), PC, axis=0)
    return [
        {
            "output": output[i * B_LOC:(i + 1) * B_LOC],
            "target": target[i * B_LOC:(i + 1) * B_LOC],
            "wconst": wconst,
        }
        for i in range(N_CORES)
    ]


def kernel(output, target, target_weights):
    from concourse.bass_utils import run_bass_kernel_spmd

    nc = _get_nc()
    in_maps = make_in_maps(output, target)
    res = run_bass_kernel_spmd(nc, in_maps, list(range(N_CORES)))
    parts = np.stack([res.results[i]["partials"] for i in range(N_CORES)])
    return np.asarray(_combine(parts, target_weights), dtype=np.float32)


# revision 44
# speedup vs baseline: 1.2837x; 1.2837x over previous
"""Trainium2 Bass kernel for nn_CombinedTargetIOULoss (B=64, K=17, H=W=64).

Data-parallel over batch: 8 cores x 8 batches each. Per core the kernel
computes per-(b,k) partial sums [sum(q1+q2), sum((hp-hg)^2)] over the 4096
pixels; the host combines them into the scalar loss (incl. target-weight
scaling and the tw==0 mask case).

Key algebra (pixel anchors cancel out of the reference box math):
  dx = |p-g|, sx = |p|+|g|; u2 = sx-dx = 2*iw, v2 = sx+dx = 2*cw (same y)
  inter = it4/4 (it4 = u2x*u2y), area_c = ac4/4, t1 = |p||q|, t2 = |g||h|
  union = t1+t2+EPS-it4/4;  q1 = inter/union, q2 = union/area_c
  Scaled to dodge divisions by 4:
    ue4 = 4(t1+t2+EPS) - it4        -> q1 = it4 * (1/ue4)
    ac4e = ac4 + 4*EPS              -> q2 = ue4 * (1/ac4e)
  (ue4 >= 2(t1+t2) since inter <= (area_p+area_g)/2 -> no cancellation.)

Layout: partition = (b, pc) with pc = p//256; the host pre-transposes the
inputs to [b, pc, kc, s] so every per-chunk DMA is a fully contiguous
[128, kg*3*256] block (multi-KB descriptors; the old 256B descriptors paid
the <512B 2x DMA latency penalty). Components (hm/ox/oy) sit at different
free offsets of the same partition, so cross-component elementwise ops
stay partition-aligned.

Engine split (ASSIGN below; DMA ~37-41us/core is the roofline):
  Pool: raw f32 diff + the sxy/u2/v2 box sums (tensor_tensor)
  ACT : |ox|,|oy| of both tensors, |diff_xy|, diff_hm^2 (unary, any dtype)
  DVE : bf16 products (tensor_tensor = 2x perf mode), the two
        RECIPROCAL_APPROX_FAST, and the tensor_scalar affines (4x mode)
  PE  : pixel reduction: 16 psum-accumulating one-hot matmuls per
        (tensor, chunk) fold s and (b,pc)->b at once; PE is otherwise idle
All cross-engine sync is derived automatically by a writer/reader tracker
(standalone wait_ge on monotone per-engine counters; per-DMA semaphores
because DMA completion order across transfers is not guaranteed).
"""

import sys

sys.path.insert(0, "/opt/trn_rl_repo")

import numpy as np

import concourse.bass as bass
from concourse import mybir
from concourse.alu_op_type import AluOpType as Alu

F32 = mybir.dt.float32
BF16 = mybir.dt.bfloat16
AF = mybir.ActivationFunctionType

EPS = 1e-7
B, K, H, W = 64, 17, 64, 64
C = 3 * K
P = H * W
N_CORES = 8
B_LOC = B // N_CORES          # 8 batches per core
PC = 16                       # pixel chunks per batch -> partition = (b, pc)
S = P // PC                   # 256 pixels per partition line per (k, c)
NSI = 16                      # matmul si-slices folding s

CH = [2, 4, 4, 4, 2, 1]       # k-group chunk sizes (sum = 17)
KMAX = max(CH)
NCH = len(CH)
# SWDGE (gpsimd-dispatched) DMA casts f32->bf16 in flight: halves SBUF
# write traffic and makes every eng op see 2-byte operands (DVE 2x/4x).
CAST_DMA = True

# op -> engine stream ('gp' = Pool, 'act' = Scalar, 'dve' = Vector)
ASSIGN = {
    "diff": "dve", "sxy": "dve", "u2": "dve", "v2": "dve",
    "oabs": "act", "tabs": "act", "dxy": "act", "dsq": "act",
    "x1m": "act", "ac4e": "act",
    "t1": "dve", "t2": "dve", "it4": "dve", "ac4": "dve", "x1": "dve",
    "ue4": "dve", "m": "dve", "n1": "dve", "n2": "dve", "n": "dve",
    "qsP": "dve", "tr1": "dve", "tr2": "dve", "tr3": "dve", "tr4": "dve",
}


class _Waiter:
    """Dedupe monotone standalone waits per (engine, sem)."""

    def __init__(self):
        self.seen = {}

    def wait(self, eng, sem, val):
        if val <= 0:
            return
        key = (id(eng), sem.name if hasattr(sem, "name") else id(sem))
        if self.seen.get(key, -1) >= val:
            return
        self.seen[key] = val
        eng.wait_ge(sem, val)


class _Tracker:
    """Auto-emit cross-engine waits from tile writer/reader records.

    Tiles are slot-granular keys. Same-stream dependencies rely on the
    engine's in-order execution and emit nothing.
    """

    def __init__(self, wt):
        self.wt = wt
        self.tiles = {}   # key -> {"w": (stream, sem, cnt) | None,
                          #         "r": {stream: (sem, cnt)}}

    def _rec(self, key):
        return self.tiles.setdefault(key, {"w": None, "r": {}})

    def pre(self, stream, eng, ins, outs):
        for key in ins:
            w = self._rec(key)["w"]
            if w and w[0] != stream:
                self.wt.wait(eng, w[1], w[2])
        for key in outs:
            rec = self._rec(key)
            w = rec["w"]
            if w and w[0] != stream:
                self.wt.wait(eng, w[1], w[2])
            for rs, (rsem, rcnt) in rec["r"].items():
                if rs != stream:
                    self.wt.wait(eng, rsem, rcnt)

    def post(self, stream, sem, cnt, ins, outs):
        for key in ins:
            self._rec(key)["r"][stream] = (sem, cnt)
        for key in outs:
            rec = self._rec(key)
            rec["w"] = (stream, sem, cnt)
            rec["r"] = {}


def _build_body(nc, o_ext, t_ext, w_ext, p_ext):
    from concourse.dve_ops import (
        RECIP_APPROX_FAST_CONSTS as RAF_C,
        RECIPROCAL_APPROX_FAST as RAF_OP,
    )

    sb = lambda name, shape, dt: nc.alloc_sbuf_tensor(name, shape, dt).ap()

    ov = o_ext.rearrange("b pc kc s -> (b pc) (kc s)")
    tv = t_ext.rearrange("b pc kc s -> (b pc) (kc s)")

    # --- SBUF tiles, all double-buffered by chunk parity ---
    def pair(name, ncomp, dt=BF16, s=S):
        ts = [sb(f"{name}{i}", [128, KMAX * ncomp * s], dt) for i in range(2)]
        return [t.rearrange("p (k c s) -> p k c s", k=KMAX, c=ncomp, s=s)
                for t in ts]

    IND = BF16 if CAST_DMA else F32
    in2 = [sb(f"in2{i}", [128, 2 * KMAX * 3 * S], IND) for i in range(2)]
    in2v = [t.rearrange("p (t k c s) -> p t k c s", t=2, k=KMAX, c=3, s=S)
            for t in in2]
    in2f = [t.rearrange("p (t x) -> p t x", t=2, x=KMAX * 3 * S) for t in in2]
    diffv = pair("df", 3)
    abov = pair("abo", 2)
    abtv = pair("abt", 2)
    dxyv = pair("dxy", 2)
    sxyv = pair("sxy", 2)
    u2v = pair("u2", 2)
    v2v = pair("v2", 2)
    one = lambda name: [t[:, :, 0] for t in pair(name, 1)]
    t1v, t2v, it4v, ac4v, x1v, x1mv, ue4v, ac4ev, mv, rv, n1v, n2v, nv = (
        one(n) for n in ("t1", "t2", "it4", "ac4", "x1", "x1m", "ue4",
                         "ac4e", "mT", "rT", "n1", "n2", "nT"))
    # Q stacks {qsP, dsq} as (k, t, s) so the PE stage sees (k, t) slices
    # that flatten contiguously; tree scratch folds s 256 -> 32
    Qts = [sb(f"Q{i}", [128, KMAX * 2 * S], BF16) for i in range(2)]
    Qv = [t.rearrange("p (k t s) -> p k t s", k=KMAX, t=2, s=S) for t in Qts]
    tAs = [sb(f"tA{i}", [128, KMAX * 2 * 128], BF16) for i in range(2)]
    tAv = [t.rearrange("p (k t s) -> p k t s", k=KMAX, t=2, s=128) for t in tAs]
    tBs = [sb(f"tB{i}", [128, KMAX * 2 * 64], BF16) for i in range(2)]
    tBv = [t.rearrange("p (k t s) -> p k t s", k=KMAX, t=2, s=64) for t in tBs]
    Q32s = [sb(f"Q32{i}", [128, KMAX * 2 * 32], BF16) for i in range(2)]
    Q32v = [t.rearrange("p (k t s) -> p k t s", k=KMAX, t=2, s=32)
            for t in Q32s]
    Q16s = [sb(f"Q16{i}", [128, KMAX * 2 * 16], BF16) for i in range(2)]
    Q16v = [t.rearrange("p (k t s) -> p k t s", k=KMAX, t=2, s=16)
            for t in Q16s]

    wts = sb("wts", [128, B_LOC], BF16)
    osb = sb("osb", [B_LOC, 2 * K], F32)
    psum = nc.alloc_psum_tensor("ps", [B_LOC, 2 * K], F32).ap()
    psumv = psum.rearrange("p (k t) -> p k t", k=K, t=2)

    # --- semaphores ---
    dma_o = [nc.alloc_semaphore(f"dma_o{j}") for j in range(NCH)]
    dma_t = [nc.alloc_semaphore(f"dma_t{j}") for j in range(NCH)]
    wdma = nc.alloc_semaphore("wdma")
    dma_out = nc.alloc_semaphore("dma_out")
    act_c = nc.alloc_semaphore("act_c")
    dve_c = nc.alloc_semaphore("dve_c")
    gp_c = nc.alloc_semaphore("gp_c")
    pe_c = nc.alloc_semaphore("pe_c")
    wt = _Waiter()
    tk = _Tracker(wt)

    ENG = {"gp": (nc.gpsimd, gp_c), "act": (nc.scalar, act_c),
           "dve": (nc.vector, dve_c), "pe": (nc.tensor, pe_c)}
    cnt = {"gp": 0, "act": 0, "dve": 0, "pe": 0}

    def run(stream, ins, outs, emit):
        eng, sem = ENG[stream]
        tk.pre(stream, eng, ins, outs)
        emit(eng).then_inc(sem, 1)
        cnt[stream] += 1
        tk.post(stream, sem, cnt[stream], ins, outs)

    # binary op via the op->engine table
    def tt(name, out_ap, out_key, a_ap, a_key, b_ap, b_key, op):
        st = ASSIGN[name]
        run(st, [a_key, b_key], [out_key],
            lambda eng: eng.tensor_tensor(out_ap, a_ap, b_ap, op))

    # unary: ACT activation or DVE tensor_scalar (4x perf mode)
    def un(name, out_ap, out_key, in_ap, in_key, func, ts_args, **kw):
        st = ASSIGN[name]
        if st == "act":
            run(st, [in_key], [out_key],
                lambda eng: nc.scalar.activation(out_ap, in_ap, func, **kw))
        else:
            s1, s2, op0, op1 = ts_args
            if op1 is Ellipsis:
                run("dve", [in_key], [out_key],
                    lambda eng: eng.tensor_scalar(out_ap, in_ap, s1, s2, op0))
            else:
                run("dve", [in_key], [out_key],
                    lambda eng: eng.tensor_scalar(out_ap, in_ap, s1, s2,
                                                  op0, op1))

    # --- one-hot PE weights: SWDGE (gpsimd) DMA casts f32 -> bf16 ---
    nc.gpsimd.dma_start(out=wts[:], in_=w_ext[:]).then_inc(wdma, 16)
    tk.post("dma", wdma, 16, [], [("wts",)])

    K0 = [sum(CH[:i]) for i in range(NCH)]

    dma_eng, dma_stream = ((nc.gpsimd, "gp") if CAST_DMA
                           else (nc.sync, "sync"))

    def emit_dma(j):
        kg, k0 = CH[j], K0[j]
        sl = j % 2
        cs0, csn = 3 * k0 * S, 3 * kg * S
        tk.pre(dma_stream, dma_eng, [], [("ino", sl)])
        dma_eng.dma_start(
            out=in2f[sl][:, 0, 0:csn], in_=ov[:, cs0:cs0 + csn]
        ).then_inc(dma_o[j], 16)
        tk.post("dma", dma_o[j], 16, [], [("ino", sl)])
        tk.pre(dma_stream, dma_eng, [], [("int", sl)])
        dma_eng.dma_start(
            out=in2f[sl][:, 1, 0:csn], in_=tv[:, cs0:cs0 + csn]
        ).then_inc(dma_t[j], 16)
        tk.post("dma", dma_t[j], 16, [], [("int", sl)])

    def emit_diff(j):
        kg, sl = CH[j], j % 2
        tt("diff", diffv[sl][:, 0:kg], ("df", sl),
           in2v[sl][:, 0, 0:kg], ("ino", sl),
           in2v[sl][:, 1, 0:kg], ("int", sl), Alu.subtract)

    def emit_abs(j):
        kg, sl = CH[j], j % 2
        un("oabs", abov[sl][:, 0:kg], ("abo", sl),
           in2v[sl][:, 0, 0:kg, 1:3], ("ino", sl), AF.Abs,
           (0.0, None, Alu.abs_max, Ellipsis))
        un("tabs", abtv[sl][:, 0:kg], ("abt", sl),
           in2v[sl][:, 1, 0:kg, 1:3], ("int", sl), AF.Abs,
           (0.0, None, Alu.abs_max, Ellipsis))
        un("dxy", dxyv[sl][:, 0:kg], ("dxy", sl),
           diffv[sl][:, 0:kg, 1:3], ("df", sl), AF.Abs,
           (0.0, None, Alu.abs_max, Ellipsis))
        if ASSIGN["dsq"] == "act":
            un("dsq", Qv[sl][:, 0:kg, 1], ("Q", sl),
               diffv[sl][:, 0:kg, 0], ("df", sl), AF.Square,
               (None, None, None, None))
        else:
            tt("dsq", Qv[sl][:, 0:kg, 1], ("Q", sl),
               diffv[sl][:, 0:kg, 0], ("df", sl),
               diffv[sl][:, 0:kg, 0], ("df", sl), Alu.mult)

    def emit_box(j):
        kg, sl = CH[j], j % 2
        tt("sxy", sxyv[sl][:, 0:kg], ("sxy", sl),
           abov[sl][:, 0:kg], ("abo", sl),
           abtv[sl][:, 0:kg], ("abt", sl), Alu.add)
        tt("u2", u2v[sl][:, 0:kg], ("u2", sl),
           sxyv[sl][:, 0:kg], ("sxy", sl),
           dxyv[sl][:, 0:kg], ("dxy", sl), Alu.subtract)
        tt("v2", v2v[sl][:, 0:kg], ("v2", sl),
           sxyv[sl][:, 0:kg], ("sxy", sl),
           dxyv[sl][:, 0:kg], ("dxy", sl), Alu.add)

    def emit_dve_a(j):
        kg, sl = CH[j], j % 2
        tt("t1", t1v[sl][:, 0:kg], ("t1", sl),
           abov[sl][:, 0:kg, 0], ("abo", sl),
           abov[sl][:, 0:kg, 1], ("abo", sl), Alu.mult)
        tt("t2", t2v[sl][:, 0:kg], ("t2", sl),
           abtv[sl][:, 0:kg, 0], ("abt", sl),
           abtv[sl][:, 0:kg, 1], ("abt", sl), Alu.mult)
        tt("it4", it4v[sl][:, 0:kg], ("it4", sl),
           u2v[sl][:, 0:kg, 0], ("u2", sl),
           u2v[sl][:, 0:kg, 1], ("u2", sl), Alu.mult)
        tt("ac4", ac4v[sl][:, 0:kg], ("ac4t", sl),
           v2v[sl][:, 0:kg, 0], ("v2", sl),
           v2v[sl][:, 0:kg, 1], ("v2", sl), Alu.mult)
        tt("x1", x1v[sl][:, 0:kg], ("x1", sl),
           t1v[sl][:, 0:kg], ("t1", sl),
           t2v[sl][:, 0:kg], ("t2", sl), Alu.add)

    def emit_act_affine(j):
        kg, sl = CH[j], j % 2
        # ac4e = ac4 + 4eps;  x1m = 4*x1 + 4eps (= 4(t1+t2+eps))
        un("ac4e", ac4ev[sl][:, 0:kg], ("ac4e", sl),
           ac4v[sl][:, 0:kg], ("ac4t", sl), AF.Copy,
           (1.0, 4.0 * EPS, Alu.mult, Alu.add), scale=1.0, bias=4.0 * EPS)
        un("x1m", x1mv[sl][:, 0:kg], ("x1m", sl),
           x1v[sl][:, 0:kg], ("x1", sl), AF.Copy,
           (4.0, 4.0 * EPS, Alu.mult, Alu.add), scale=4.0, bias=4.0 * EPS)

    def emit_dve_b(j):
        kg, sl = CH[j], j % 2
        # qs + 1 = (x1m*ac4e + ue4^2) / (ue4*ac4e); host subtracts the +1
        tt("ue4", ue4v[sl][:, 0:kg], ("ue4", sl),
           x1mv[sl][:, 0:kg], ("x1m", sl),
           it4v[sl][:, 0:kg], ("it4", sl), Alu.subtract)
        tt("m", mv[sl][:, 0:kg], ("m", sl),
           ue4v[sl][:, 0:kg], ("ue4", sl),
           ac4ev[sl][:, 0:kg], ("ac4e", sl), Alu.mult)
        run("dve", [("m", sl)], [("r", sl)],
            lambda eng: eng._custom_dve(
                RAF_OP, out=rv[sl][:, 0:kg], in0=mv[sl][:, 0:kg],
                s0=RAF_C["s0"], s1=RAF_C["s1"], imm2=RAF_C["imm2"]))
        tt("n1", n1v[sl][:, 0:kg], ("n1", sl),
           x1mv[sl][:, 0:kg], ("x1m", sl),
           ac4ev[sl][:, 0:kg], ("ac4e", sl), Alu.mult)
        tt("n2", n2v[sl][:, 0:kg], ("n2", sl),
           ue4v[sl][:, 0:kg], ("ue4", sl),
           ue4v[sl][:, 0:kg], ("ue4", sl), Alu.mult)
        tt("n", nv[sl][:, 0:kg], ("n", sl),
           n1v[sl][:, 0:kg], ("n1", sl),
           n2v[sl][:, 0:kg], ("n2", sl), Alu.add)
        tt("qsP", Qv[sl][:, 0:kg, 0], ("Q", sl),
           nv[sl][:, 0:kg], ("n", sl),
           rv[sl][:, 0:kg], ("r", sl), Alu.mult)
        # pairwise tree folds s 256 -> 32 for both Q halves at once
        tt("tr1", tAv[sl][:, 0:kg], ("tA", sl),
           Qv[sl][:, 0:kg, :, 0:128], ("Q", sl),
           Qv[sl][:, 0:kg, :, 128:256], ("Q", sl), Alu.add)
        tt("tr2", tBv[sl][:, 0:kg], ("tB", sl),
           tAv[sl][:, 0:kg, :, 0:64], ("tA", sl),
           tAv[sl][:, 0:kg, :, 64:128], ("tA", sl), Alu.add)
        tt("tr3", Q32v[sl][:, 0:kg], ("Q32", sl),
           tBv[sl][:, 0:kg, :, 0:32], ("tB", sl),
           tBv[sl][:, 0:kg, :, 32:64], ("tB", sl), Alu.add)
        tt("tr4", Q16v[sl][:, 0:kg], ("Q16", sl),
           Q32v[sl][:, 0:kg, :, 0:16], ("Q32", sl),
           Q32v[sl][:, 0:kg, :, 16:32], ("Q32", sl), Alu.add)

    def emit_pe(j):
        kg, k0, sl = CH[j], K0[j], j % 2
        # fold the remaining s=32 and (b,pc)->b at once: 32 matmuls
        # accumulate psum[:, :, k0:k1] += wts^T @ Q32[:, :, :, si]
        eng, sem = ENG["pe"]
        tk.pre("pe", eng, [("Q16", sl), ("wts",)], [])
        for si in range(16):
            nc.tensor.matmul(
                psumv[:, k0:k0 + kg], wts[:],
                Q16v[sl][:, 0:kg, :, si],
                start=(si == 0), stop=(si == 15),
            ).then_inc(pe_c, 1)
            cnt["pe"] += 1
        tk.post("pe", sem, cnt["pe"], [("Q16", sl), ("wts",)], [])

    # --- software-pipelined emission (1-chunk skew; diff_j sits mid-way
    # through the j-1 DVE group so DVE never head-blocks on fresh DMA) ---
    for j in range(NCH):
        emit_dma(j)
        if j >= 1:
            emit_box(j - 1)
            emit_dve_a(j - 1)
        emit_diff(j)
        if j >= 1:
            emit_act_affine(j - 1)
            emit_dve_b(j - 1)
            emit_pe(j - 1)
        emit_abs(j)
    emit_box(NCH - 1)
    emit_dve_a(NCH - 1)
    emit_act_affine(NCH - 1)
    emit_dve_b(NCH - 1)
    emit_pe(NCH - 1)

    # --- tail: psum -> sbuf -> dram ---
    wt.wait(nc.vector, pe_c, cnt["pe"])
    run("dve", [], [("osb",)],
        lambda eng: eng.tensor_copy(osb[:], psum[:]))
    tk.pre("sync", nc.sync, [("osb",)], [])
    nc.sync.dma_start(out=p_ext[:], in_=osb[:]).then_inc(dma_out, 16)
    nc.sync.wait_ge(dma_out, 16)


def build_nc():
    nc = bass.Bass()
    o_ext = nc.declare_dram_parameter("output", [B_LOC, PC, C, S], F32,
                                      isOutput=False)
    t_ext = nc.declare_dram_parameter("target", [B_LOC, PC, C, S], F32,
                                      isOutput=False)
    w_ext = nc.declare_dram_parameter("wconst", [128, B_LOC], F32,
                                      isOutput=False)
    p_ext = nc.declare_dram_parameter("partials", [B_LOC, 2 * K], F32,
                                      isOutput=True)
    _build_body(nc, o_ext, t_ext, w_ext, p_ext)
    # fill the 64-byte ISA encodings of custom DVE ops (reciprocal_approx):
    # Bacc.compile() does this; the raw-Bass + PJRT path does not.
    mybir.codegen_inst_isa_subclasses(nc)
    return nc


_NC = None


def _get_nc():
    global _NC
    if _NC is None:
        _NC = build_nc()
    return _NC


def _combine(parts, target_weights):
    """parts: [8 cores, 8, 34] f32 -> scalar loss (host-side finish)."""
    arr = np.asarray(parts, np.float64).reshape(B, K, 2)
    sqs = arr[:, :, 0]      # sum over pixels of (q1 + q2 + 1), per (b, k)
    ssd = arr[:, :, 1]      # sum over pixels of (hp - hg)^2, per (b, k)

    tw = np.asarray(target_weights, np.float64)
    twnz = (tw != 0).astype(np.float64)
    # kernel stores sum(q1+q2+1) so the giou term is 2P - (sqs - P)
    num = ((3.0 * P - sqs) * twnz).sum(axis=0)
    den = np.maximum((P * twnz).sum(axis=0), 1.0)
    giou_joint = num / den
    mse = 0.5 * (tw**2 * ssd).sum(axis=0) / (B * P)
    return np.float32(np.sum(mse + giou_joint) / K)


def make_in_maps(output, target):
    """Host-side relayout [B, C, P] -> [B, pc, C, s] (contiguous DMA blocks)
    plus the one-hot fold matrix, split per core."""
    output = np.ascontiguousarray(
        np.asarray(output, dtype=np.float32).reshape(B, C, PC, S)
        .transpose(0, 2, 1, 3))
    target = np.ascontiguousarray(
        np.asarray(target, dtype=np.float32).reshape(B, C, PC, S)
        .transpose(0, 2, 1, 3))
    wconst = np.repeat(np.eye(B_LOC, dtype=np.float32), PC, axis=0)
    return [
        {
            "output": output[i * B_LOC:(i + 1) * B_LOC],
            "target": target[i * B_LOC:(i + 1) * B_LOC],
            "wconst": wconst,
        }
        for i in range(N_CORES)
    ]


def kernel(output, target, target_weights):
    from concourse.bass_utils import run_bass_kernel_spmd

    nc = _get_nc()
    in_maps = make_in_maps(output, target)
    res = run_bass_kernel_spmd(nc, in_maps, list(range(N_CORES)))
    parts = np.stack([res.results[i]["partials"] for i in range(N_CORES)])
    return np.asarray(_combine(parts, target_weights), dtype=np.float32)
